# revision 65
# baseline (speedup 1.0000x reference)
"""Causal GQA attention for Trainium2, sharded across 8 NeuronCores.

Problem: q [2, 2048, 32, 128], k/v [2, 2048, 8, 128] fp32, causal,
GQA group = 4. Sharding: core i gets kv-head i plus its 4 q-heads
(heads 4i..4i+3), both batch elements. Each core runs the same program
(SPMD) on its shard; outputs are concatenated on the head axis.

v2 design (fp16, flat pipeline, engine-balanced; ~178us/run on HW vs
243us f32r baseline; TimelineSim predicts 168us with PE and ACT both
~154us busy - the twin walls of this decomposition):
  All matmul operands are fp16 (host pre-converts; 10-bit mantissa
  keeps rel err ~6e-4, and fp16 streams 1 col/cycle on PE at any
  width, so causal diagonal chunks need no widening).  The exp runs on
  ACT over chunk PAIRS (one instruction per two k-chunks, psum tile
  [128,2,512] spanning 2 banks) to halve the ~190ns/instr access
  bubble.  Causal triangles are zeroed post-exp on the Pool engine
  (affine_select).  The denominator: non-diagonal chunk pairs are
  pre-summed on DVE (fp16 2x mode) so the ones-matmul runs once per
  pair (DGROUP=2), cutting D's PE cols by a third.  Normalization is
  NOT done on device: the kernel ships unnormalized O (fp16) plus the
  denominator row (f32), and the host divides - this removed a ~140us
  serialization (reciprocal+mul on DVE reading single-buffered psum).
  exp gets a -4 bias (cancels in the division) so unnormalized O stays
  in fp16 range.  One flat software pipeline runs over all
  (head, q-tile, pair) items with S-pairs PIPE_P=3 ahead of O/D -
  no drain bursts at tile/head boundaries; the last head runs its
  q-tiles largest-first and output slices store per q-tile on the SP
  queue, keeping the final drain short.  PSUM: 2 S-pair tiles (4
  banks) + O x2 + D x2 = 8.

The host pre-arranges shards so the device does zero transposes or
dtype converts: q arrives [B, HL, D, QL] fp16, k [B, D, KL] fp16,
v [B, 128, KL/128, 128] fp16 (chunk-swizzled); out leaves as
[B, HL, D, QL] fp16 + den [B, HL, 1, QL] f32 (host divides,
transposes, casts back).
"""

import math

import numpy as np

import concourse.tile as tile
from concourse import bacc, mybir
from concourse.bass_utils import run_bass_kernel_spmd

P = 128
F32 = mybir.dt.float32
F16 = mybir.dt.float16

ABLATE = set()        # timing-only ablations (wrong numerics when set)
PIPE_P = 4            # S-pair pipeline depth ahead of O/D
MASK_MODE = "pool_select"  # pool_select | dve_mul | pool_mul
DGROUP = 8            # denominator folding: 1 = D matmul per chunk;
                      # 2 = per non-diag pair (engine pre-sum); 3 = also
                      # fold diag pairs; 4 = quad-fold non-diag pairs;
                      # 8 = octet-fold non-diag pairs
DSUM_ENGINE = "dve"   # dve | pool: engine for DGROUP=2 pair sums
OD_ORDER = "interleave"  # unused in flat pipeline (kept for bench compat)
NORM_MODE = "host"    # host: ship unnormalized O + den row, divide on host
                      # device: reciprocal+mul per qt on DVE
                      # (DMA cannot read PSUM, so both stage via SBUF)
BUFS = {"ps_s": 2, "ps_o": 2, "ps_d": 2, "sb_pt": 9}


def emit_attention(nc, tc, ctx, q_ap, k_ap, v_ap, o_ap, d_ap, B, QL, KL, HL,
                   D):
    """q_ap/o_ap: [B, HL, D, QL] f16; k_ap: [B, D, KL] f16;
    v_ap: [B, P, KL/P, P] f16; d_ap: [B, HL, QL] f16 (denominator row)."""
    assert D == P
    QT = 512                       # q tile (psum bank = 512 fp32)
    KC = P                         # k chunk
    n_qt = QL // QT
    n_kc_total = KL // KC
    scale = 1.0 / math.sqrt(D)

    sb = ctx.enter_context(tc.tile_pool(name="sb", bufs=1))
    sb_q = ctx.enter_context(tc.tile_pool(name="sb_q", bufs=BUFS.get("sb_q", 3)))
    sb_pt = ctx.enter_context(tc.tile_pool(name="sb_pt", bufs=BUFS["sb_pt"]))
    sb_o = ctx.enter_context(tc.tile_pool(name="sb_o", bufs=3))
    ps_s = ctx.enter_context(
        tc.tile_pool(name="ps_s", bufs=BUFS["ps_s"], space="PSUM"))
    ps_o = ctx.enter_context(
        tc.tile_pool(name="ps_o", bufs=BUFS["ps_o"], space="PSUM"))
    ps_d = ctx.enter_context(
        tc.tile_pool(name="ps_d", bufs=BUFS["ps_d"], space="PSUM"))

    ones_f32 = sb.tile([P, P], F32, name="ones_f32")
    nc.gpsimd.memset(ones_f32[:], 1.0)
    ones = sb.tile([P, P], F16, name="ones")
    nc.vector.tensor_copy(ones[:], ones_f32[:])
    # touch Exp at t=0 so the ~1.3us ACT table load runs during the
    # preamble DMAs instead of on the first real exp's critical path
    actwarm = sb.tile([1, 1], F32, name="actwarm")
    nc.scalar.activation(actwarm[:], ones_f32[:1, :1],
                         mybir.ActivationFunctionType.Exp)
    # per-partition exp bias: -4 keeps unnormalized O / den in fp16 range
    ebias = sb.tile([P, 1], F32, name="ebias")
    nc.gpsimd.memset(ebias[:], -4.0)
    # 0/1 causal mask in S^T coords: keep where q_local >= k_local
    mask01 = sb.tile([P, P], F16, name="mask01")
    nc.gpsimd.memset(mask01[:], 1.0)
    nc.gpsimd.affine_select(
        out=mask01[:], in_=mask01[:], compare_op=mybir.AluOpType.is_ge,
        fill=0.0, base=0, pattern=[[1, P]], channel_multiplier=-1)

    # --- K/V preamble; first S-matmul's operands land first ---
    KTs, Vs = [], []
    for b in range(B):
        KTs.append(sb.tile([P, KL], F16, name=f"KT{b}"))
        Vs.append(sb.tile([P, n_kc_total, P], F16, name=f"V{b}"))

    def emit_qload(b, h, split=False):
        QTt = sb_q.tile([P, QL], F16, tag="qtt")
        if split:
            nc.sync.dma_start(QTt[:, :QT], q_ap[b, h][:, :QT])
            nc.sync.dma_start(QTt[:, QT:], q_ap[b, h][:, QT:])
        else:
            nc.sync.dma_start(QTt[:], q_ap[b, h])
        return QTt

    G = 4
    gk = KL // G
    gc = n_kc_total // G
    # first S-pair needs K chunks 0-1 and Q cols [0:512): land those first
    nc.sync.dma_start(KTs[0][:, :2 * KC], k_ap[0][:, :2 * KC])
    qtt_cur = emit_qload(0, 0, split=True)
    nc.sync.dma_start(KTs[0][:, 2 * KC:gk], k_ap[0][:, 2 * KC:gk])
    nc.sync.dma_start(Vs[0][:, :gc, :], v_ap[0][:, :gc, :])
    for b in range(B):
        for g in range(G):
            if b == 0 and g == 0:
                continue
            nc.sync.dma_start(KTs[b][:, g * gk:(g + 1) * gk],
                              k_ap[b][:, g * gk:(g + 1) * gk])
            nc.sync.dma_start(Vs[b][:, g * gc:(g + 1) * gc, :],
                              v_ap[b][:, g * gc:(g + 1) * gc, :])

    # --- main loop: one flat software pipeline over all (bh, qt, pair)
    # items; S-pairs run PIPE_P ahead of O/D with no drain bursts at
    # qt/bh boundaries. PSUM: ps_s 2x2 + ps_o 2 + ps_d 2 = 8 banks.
    heads = [(b, h) for b in range(B) for h in range(HL)]
    qtt_next = None
    obh_next = None
    dbh_next = None
    obh_cur = sb_o.tile([P, QL], F16, tag="obh")
    dbh_cur = sb_o.tile([1, QL], F32, tag="dbh")

    class QtState:
        pass

    def new_qt_state(bi, qt):
        st = QtState()
        st.bi, st.qt = bi, qt
        st.b, st.h = heads[bi]
        st.q0 = qt * QT
        st.n_kc = (st.q0 + QT) // KC
        st.QTt, st.Obh, st.Dbh = qtt_cur, obh_cur, dbh_cur
        O_ps = ps_o.tile([P, QT], F32, tag="o")
        st.O_ps = O_ps
        if "den" not in ABLATE:
            D_ps = ps_d.tile([P, QT], F32, tag="d")
            st.D_ps = D_ps
        else:
            st.D_ps = None
        st.pts = {}
        return st

    def emit_S_pair(st, pc):
        qt, b = st.qt, st.b
        kc0, kc1 = 2 * pc, 2 * pc + 1
        dj0, dj1 = kc0 - qt * (QT // KC), kc1 - qt * (QT // KC)
        c00 = dj0 * KC if dj0 >= 0 else 0
        c01 = dj1 * KC if dj1 >= 0 else 0
        S2 = ps_s.tile([P, 2, QT], F32, tag="s")
        for half, (kc, c) in enumerate(((kc0, c00), (kc1, c01))):
            nc.tensor.matmul(
                S2[:, half, c:], KTs[b][:, kc * KC:(kc + 1) * KC],
                st.QTt[:, st.q0 + c:st.q0 + QT], start=True, stop=True,
                skip_group_check=True)
        PT2 = sb_pt.tile([P, 2, QT], F16, tag="pt")
        # bias -4 keeps the unnormalized O / den in fp16 range; it
        # cancels in the host-side division
        ce = (c00 + QT) // 2 if "exphalf" in ABLATE else c00
        nc.scalar.activation(
            PT2[:, :, ce:], S2[:, :, ce:],
            mybir.ActivationFunctionType.Exp, scale=scale, bias=ebias[:])
        # half-1 cols [c00:c01) hold exp(stale psum) but are never
        # read: O/D matmuls for kc1 start at c01.
        if "mask" not in ABLATE:
            for half, (dj, c) in enumerate(((dj0, c00), (dj1, c01))):
                if dj >= 0:
                    # zero the strict lower triangle (q_local < k_local)
                    pt = PT2[:, half, c:c + P]
                    if MASK_MODE == "pool_select":
                        nc.gpsimd.affine_select(
                            out=pt, in_=pt,
                            compare_op=mybir.AluOpType.is_ge,
                            fill=0.0, base=0, pattern=[[1, P]],
                            channel_multiplier=-1)
                    elif MASK_MODE == "dve_mul":
                        nc.vector.tensor_mul(pt, pt, mask01[:])
                    else:
                        nc.gpsimd.tensor_mul(pt, pt, mask01[:])
        if DGROUP >= 2 and (DGROUP >= 3 or (c00 == 0 and c01 == 0)):
            # pre-sum the pair over [c00:512) so D needs ONE matmul per
            # pair; for diag pairs, half-1's gap [c00:c01) holds
            # exp(stale psum) - zero it on the idle Pool engine first
            sPT = sb_pt.tile([P, QT], F16, tag="spt")
            eng = nc.vector if DSUM_ENGINE == "dve" else nc.gpsimd
            if c01 > c00:
                nc.gpsimd.memset(PT2[:, 1, c00:c01], 0.0)
            eng.tensor_add(sPT[:, c00:], PT2[:, 0, c00:], PT2[:, 1, c00:])
            st.pts[kc0] = (PT2[:, 0, :], c00, sPT, False, c00)
            st.pts[kc1] = (PT2[:, 1, :], c01, None, True, c00)
            if DGROUP >= 4 and c00 == 0 and c01 == 0:
                # second-level fold: one D matmul per 4 non-diag chunks
                prev = getattr(st, "last_nondiag", None)
                if prev is not None and prev[0] == kc0 - 2:
                    sQ = sb_pt.tile([P, QT], F16, tag="sq")
                    eng.tensor_add(sQ[:], prev[1][:], sPT[:])
                    st.pts[prev[0]] = (st.pts[prev[0]][0], 0, sQ, False, 0)
                    st.pts[kc0] = (PT2[:, 0, :], 0, None, True, 0)
                    st.last_nondiag = None
                    qk = prev[0]
                    pq = getattr(st, "last_quad", None)
                    if DGROUP >= 8 and pq is not None and pq[0] == qk - 4:
                        # third-level fold: one D matmul per 8 chunks
                        sO = sb_pt.tile([P, QT], F16, tag="so")
                        eng.tensor_add(sO[:], pq[1][:], sQ[:])
                        st.pts[pq[0]] = (st.pts[pq[0]][0], 0, sO, False, 0)
                        st.pts[qk] = (st.pts[qk][0], 0, None, True, 0)
                        st.last_quad = None
                    elif DGROUP >= 8:
                        st.last_quad = (qk, sQ)
                else:
                    st.last_nondiag = (kc0, sPT)
        else:
            st.pts[kc0] = (PT2[:, 0, :], c00, None, False, c01)
            st.pts[kc1] = (PT2[:, 1, :], c01, None, False, c01)

    def emit_O(st, kc):
        PT, c = st.pts[kc][0], st.pts[kc][1]
        nc.tensor.matmul(
            st.O_ps[:, c:], Vs[st.b][:, kc, :], PT[:, c:],
            start=kc == 0, stop=kc == st.n_kc - 1, skip_group_check=True)

    def emit_D(st, kc):
        PT, c, sPT, folded, c1 = st.pts.pop(kc)
        if "den" in ABLATE or folded:
            return
        last = kc + 1 == st.n_kc - 1
        if sPT is not None:
            # pair-summed: one D matmul covers this chunk and the next.
            # PSUM zero regions are bank-sized, so the qt's group gets
            # exactly ONE start=True (first-emitted matmul); later
            # writes into still-pending bytes write through the zero.
            if c < c1 and kc == 0:
                # diag pair at kc0=0: narrow goes first and owns start
                nc.tensor.matmul(
                    st.D_ps[:, c:c1], ones[:, :], PT[:, c:c1],
                    start=True, stop=last, skip_group_check=True)
                nc.tensor.matmul(
                    st.D_ps[:, c1:], ones[:, :], sPT[:, c1:],
                    start=False, stop=last, skip_group_check=True)
                return
            nc.tensor.matmul(
                st.D_ps[:, c1:], ones[:, :], sPT[:, c1:],
                start=kc == 0, stop=last, skip_group_check=True)
            if c < c1:
                # diag pair mid-qt: chunk kc0's exclusive columns
                nc.tensor.matmul(
                    st.D_ps[:, c:c1], ones[:, :], PT[:, c:c1],
                    start=False, stop=last, skip_group_check=True)
        else:
            nc.tensor.matmul(
                st.D_ps[:, c:], ones[:, :], PT[:, c:],
                start=kc == 0, stop=kc == st.n_kc - 1,
                skip_group_check=True)

    def emit_OD_pair(st, pc):
        emit_O(st, 2 * pc)
        emit_O(st, 2 * pc + 1)
        emit_D(st, 2 * pc)
        emit_D(st, 2 * pc + 1)
        if 2 * pc + 1 == st.n_kc - 1:
            emit_norm(st)
            emit_store(st)

    def emit_norm(st):
        if NORM_MODE == "host_dma":
            # no SBUF staging: DMA psum straight to HBM on the SP queue
            q0 = st.q0
            nc.sync.dma_start(o_ap[st.b, st.h][:, q0:q0 + QT], st.O_ps[:])
            if st.D_ps is not None:
                nc.sync.dma_start(d_ap[st.b, st.h][:, q0:q0 + QT],
                                  st.D_ps[:1, :])
        elif NORM_MODE == "host":
            nc.vector.tensor_copy(st.Obh[:, st.q0:st.q0 + QT], st.O_ps[:])
            if st.D_ps is not None:
                nc.vector.tensor_copy(st.Dbh[:, st.q0:st.q0 + QT],
                                      st.D_ps[:1, :])
        elif "norm" in ABLATE or "den" in ABLATE:
            nc.vector.tensor_copy(st.Obh[:, st.q0:st.q0 + QT], st.O_ps[:])
        else:
            den = sb_o.tile([P, QT], F32, tag="den")
            nc.vector.reciprocal(den[:], st.D_ps[:, :])
            nc.vector.tensor_mul(st.Obh[:, st.q0:st.q0 + QT], st.O_ps[:],
                                 den[:])

    def emit_store(st):
        if NORM_MODE == "host_dma":
            return  # emit_norm already DMA'd straight from PSUM
        # per-qt slice store: keeps the final drain short and spreads
        # DMA over the whole run
        q0 = st.q0
        nc.sync.dma_start(o_ap[st.b, st.h][:, q0:q0 + QT],
                          st.Obh[:, q0:q0 + QT])
        if NORM_MODE == "host" and "den" not in ABLATE:
            nc.sync.dma_start(d_ap[st.b, st.h][:, q0:q0 + QT],
                              st.Dbh[:, q0:q0 + QT])

    work = []
    for bi in range(len(heads)):
        # last bh runs its q-tiles largest-first so the final pipeline
        # drain covers the shortest qt's O/D tail
        qts = (list(range(n_qt - 1, -1, -1)) if bi == len(heads) - 1
               else list(range(n_qt)))
        for j, qt in enumerate(qts):
            for pc in range(2 * (qt + 1)):
                work.append((bi, j == 0, qt, pc))
    odq = []
    st = None
    for bi, first_qt, qt, pc in work:
        if pc == 0:
            if first_qt and bi > 0:
                qtt_cur, obh_cur, dbh_cur = qtt_next, obh_next, dbh_next
            st = new_qt_state(bi, qt)
        emit_S_pair(st, pc)
        odq.append((st, pc))
        if bi + 1 < len(heads) and first_qt and pc == 1:
            qtt_next = emit_qload(*heads[bi + 1])
            obh_next = sb_o.tile([P, QL], F16, tag="obh")
            dbh_next = sb_o.tile([1, QL], F32, tag="dbh")
        if len(odq) > PIPE_P:
            emit_OD_pair(*odq.pop(0))
    while odq:
        emit_OD_pair(*odq.pop(0))


def build_nc(B=2, QL=2048, KL=2048, HL=4, D=128, reps=1):
    nc = bacc.Bacc("TRN2", target_bir_lowering=False, debug=False,
                   num_devices=8)
    q = nc.dram_tensor("q", [B, HL, D, QL], F16, kind="ExternalInput")
    k = nc.dram_tensor("k", [B, D, KL], F16, kind="ExternalInput")
    v = nc.dram_tensor("v", [B, P, KL // P, P], F16, kind="ExternalInput")
    o = nc.dram_tensor("out", [B, HL, D, QL],
                       F32 if NORM_MODE == "host_dma" else F16,
                       kind="ExternalOutput")
    d = nc.dram_tensor("den", [B, HL, 1, QL], F32, kind="ExternalOutput")
    from contextlib import ExitStack
    with tile.TileContext(nc) as tc:
        for _ in range(reps):
            with ExitStack() as ctx:
                emit_attention(nc, tc, ctx, q.ap(), k.ap(), v.ap(), o.ap(),
                               d.ap(), B, QL, KL, HL, D)
    nc.compile()
    return nc


def shard_inputs(q, k, v, n_cores=8):
    B, QL, H, D = q.shape
    KL = k.shape[1]
    HL = H // n_cores
    q16 = q.astype(np.float16)
    k16 = k.astype(np.float16)
    v16 = v.astype(np.float16)
    in_maps = []
    for c in range(n_cores):
        in_maps.append({
            # [B, HL, D, QL]
            "q": np.ascontiguousarray(
                q16[:, :, HL * c:HL * (c + 1), :].transpose(0, 2, 3, 1)),
            # [B, D, KL]
            "k": np.ascontiguousarray(k16[:, :, c, :].transpose(0, 2, 1)),
            # [B, P, KL/P, P]: partition = position within a 128-chunk
            "v": np.ascontiguousarray(
                v16[:, :, c, :].reshape(B, KL // P, P, D).transpose(0, 2, 1, 3)),
        })
    return in_maps


_NC_CACHE = {}


def kernel(q: np.ndarray, k: np.ndarray, v: np.ndarray) -> np.ndarray:
    B, QL, H, D = q.shape
    KL, KVH = k.shape[1], k.shape[2]
    n_cores = 8
    HL = H // n_cores            # q-heads per core (4)
    assert KVH == n_cores and H == 32 and D == 128

    if "nc" not in _NC_CACHE:
        _NC_CACHE["nc"] = build_nc(B=B, QL=QL, KL=KL, HL=HL, D=D)
    nc = _NC_CACHE["nc"]

    q = np.asarray(q, dtype=np.float32)
    k = np.asarray(k, dtype=np.float32)
    v = np.asarray(v, dtype=np.float32)
    in_maps = shard_inputs(q, k, v, n_cores)
    res = run_bass_kernel_spmd(nc, in_maps, list(range(n_cores)))
    # device output: unnormalized O [B, HL, D, QL] f16 + den [B, HL, QL]
    # f16 per core; divide on host -> [B, QL, HL, D], concat heads
    outs = []
    for r in res.results:
        o = r["out"].astype(np.float32)
        den = r["den"].astype(np.float32).reshape(B, HL, 1, QL)
        outs.append((o / den).transpose(0, 3, 1, 2))
    return np.concatenate(outs, axis=2)
